# revision 19
# baseline (speedup 1.0000x reference)
"""Block-sparse attention kernel for TRN2 (8 NeuronCores, 1 head per core).

Problem: q,k,v [1, 4096, 8, 128] f32, block_mask [64,64] bool with pattern
  causal & (2-block sliding window | vertical stripe on blocks {0,1}).
Masking is block-granular (mask expanded by repeat), so active blocks are
fully dense.

Per-core strategy (one head). The host prepares fp16 operands:
  qT, kT: [128, 4096] transposed,  vt: [128, 32*129] pre-tiled V with a
  ones-column per 128-row tile, so P^T @ [V | 1] accumulates both O and
  the softmax denominators in one matmul chain.

Scores are computed TRANSPOSED (ST[k, q] = K @ Q^T) so exp(ST) directly
yields P^T — the stationary operand PV needs. No PE transposes at all.

Performance structure (vs the 44.4us v1 baseline -> ~32us):
  - Inputs land in three big SBUF tiles via chunked DMAs on BOTH HWDGE
    queues in need order (k/v stream on sync, q stream + v tail on
    scalar); Tile's subregion tracking means consumers only wait for
    the chunks they read.  The scalar-engine issue cost hides in the
    idle window before the first exp.
  - 8 dummy matmuls on scratch data (plus small fillers in the first
    loop bodies) keep the PE busy from t=0 so the HAM clock gate
    reaches 8/8 by ~3.5us and stays there (>=3.4us of continuous PE
    activity is required to un-throttle from the 1.2GHz cold clock).
  - Software pipeline: the loop body issues batch g+1's score matmuls
    + exp before batch g's PVs, so the single ACT engine (the pacing
    engine: ~13us of exp) always runs a batch ahead of its consumers.
  - Banded scores for 4 q-pair iterations accumulate in one [128,1024]
    PSUM tile (2 banks) and get ONE strided exp; matmuls and exp skip
    the dead 64-col block per 256 (q-pair t+3 never sees k{2t,2t+1}),
    and the staircase corners are zeroed with 2 strided memsets per
    batch instead of 2 per iteration.
  - Two [128,129] PV accumulators share each PSUM bank (pair tiles), so
    4 accumulation slots rotate and the PSUM->SBUF casts (one per pair,
    on DVE) never stall the PE.
  - Output is stored UNNORMALIZED as fp16 [O' | denom] tiles; the host
    divides.  Halves write traffic and removes the reciprocal chain.
    The final store is split across both queues so the tail drains in
    parallel.
Softmax skips max-subtraction: scores*scale ~ N(0,1), exp is safe
(denominators <= ~1.4e3, numerators <= ~2.5e3 — well inside fp16 range).
"""
import sys

if '/opt/trn_rl_repo' not in sys.path:
    sys.path.insert(0, '/opt/trn_rl_repo')

import numpy as np

SEQ = 4096
D = 128
BLOCK = 64
NBLK = SEQ // BLOCK
TILES = SEQ // 128           # 32 q-pair iterations
GROUPS = TILES // 4          # 8 batches (4 iterations each)
STORE_W = 4                  # iterations per output store
N_CORES = 8
N_HEADS = 8
SCALE = 1.0 / float(np.sqrt(D))
VW = 129                     # V tile width incl ones column
OW = 129                     # output tile width incl denominator column


def _expected_block_mask():
    q = np.arange(NBLK)[:, None]
    k = np.arange(NBLK)[None, :]
    causal = q >= k
    sliding = (q - k) < 2
    vert = np.zeros(NBLK, dtype=bool)
    vert[0:2] = True
    return causal & (sliding | vert[None, :])


_CACHED_NC = None


def _build_nc():
    import concourse.bass as bass
    import concourse.bacc as bacc
    import concourse.tile as tile
    import concourse.mybir as mybir

    f32 = mybir.dt.float32
    f16 = mybir.dt.float16
    Exp = mybir.ActivationFunctionType.Exp

    nc = bacc.Bacc(None, target_bir_lowering=False)

    qt_d = nc.dram_tensor("qT", [D, SEQ], f16, kind="ExternalInput")
    kt_d = nc.dram_tensor("kT", [D, SEQ], f16, kind="ExternalInput")
    v_d = nc.dram_tensor("vt", [D, TILES * VW], f16, kind="ExternalInput")
    o_d = nc.dram_tensor("o", [D, TILES * OW], f16, kind="ExternalOutput")

    with tile.TileContext(nc) as tc:
        with tc.tile_pool(name="inputs", bufs=1) as inputs, \
             tc.tile_pool(name="ptv_pool", bufs=3) as ptv_pool, \
             tc.tile_pool(name="pts_pool", bufs=3) as pts_pool, \
             tc.tile_pool(name="o_pool", bufs=5) as o_pool, \
             tc.tile_pool(name="stv_ps", bufs=2, space="PSUM") as stv_ps, \
             tc.tile_pool(name="st_ps", bufs=2, space="PSUM") as st_ps, \
             tc.tile_pool(name="o_ps", bufs=2, space="PSUM") as o_ps:

            kt = inputs.tile([128, SEQ], f16, name="kt", tag="kt")
            qt = inputs.tile([128, SEQ], f16, name="qt", tag="qt")
            vt = inputs.tile([128, TILES * VW], f16, name="vt", tag="vt")
            scr = inputs.tile([128, 512], f16, name="scr", tag="scr")

            # ---- loads: two HWDGE queues as parallel streams in need
            # order — the k stream (+ most of v) on sync, the q stream
            # (+ v tail) on scalar.  The scalar engine's issue cost all
            # lands in the idle fill phase before the first exp.  Tile
            # tracks subregions, so consumers only wait for the chunks
            # they actually read.
            nc.sync.dma_start(out=kt[:, 0:1024], in_=kt_d[:, 0:1024])
            nc.scalar.dma_start(out=qt[:, 0:576], in_=qt_d[:, 0:576])
            nc.sync.dma_start(out=vt[:, 0:516], in_=v_d[:, 0:516])
            nc.scalar.dma_start(out=qt[:, 576:1152], in_=qt_d[:, 576:1152])
            nc.sync.dma_start(out=kt[:, 1024:2048], in_=kt_d[:, 1024:2048])
            nc.scalar.dma_start(out=qt[:, 1152:2304], in_=qt_d[:, 1152:2304])
            nc.sync.dma_start(out=vt[:, 516:1032], in_=v_d[:, 516:1032])
            nc.scalar.dma_start(out=vt[:, 3096:4128], in_=v_d[:, 3096:4128])
            nc.sync.dma_start(out=vt[:, 1032:2064], in_=v_d[:, 1032:2064])
            nc.scalar.dma_start(out=qt[:, 2304:3456], in_=qt_d[:, 2304:3456])
            nc.sync.dma_start(out=kt[:, 2048:3072], in_=kt_d[:, 2048:3072])
            nc.scalar.dma_start(out=qt[:, 3456:4096], in_=qt_d[:, 3456:4096])
            nc.sync.dma_start(out=vt[:, 2064:3096], in_=v_d[:, 2064:3096])
            nc.sync.dma_start(out=kt[:, 3072:4096], in_=kt_d[:, 3072:4096])

            nc.gpsimd.memset(scr[:], 0.0)

            def vbt(t):
                return vt[:, VW * t:VW * t + VW]

            # ---- PE warm-up: dummy matmuls keep the PE busy from t=0 so
            # the HAM clock gate reaches 8/8 by ~3.5us while the first
            # input chunks are in flight.  They cycle through the stv
            # PSUM ring (write-only; in-order WAW deps are free).
            def dummies(n, w=512):
                for _ in range(n):
                    dummy = stv_ps.tile([128, 512], f32,
                                        name="dummy", tag="stv")
                    nc.tensor.matmul(dummy[:, 0:w], scr[:, 0:128],
                                     scr[:, 0:w], start=True, stop=True)

            dummies(8)

            ptvs = [None] * GROUPS
            pts_tiles = [None] * GROUPS   # [128, 1024] fp16, 4 quarters

            def make_scores(g):
                """Score matmuls + exp + corner memsets for batch g."""
                # vertical stripe for this group's 512 q columns
                stv = stv_ps.tile([128, 512], f32, name="stv", tag="stv")
                nc.tensor.matmul(stv[:], kt[:, 0:128],
                                 qt[:, 512 * g:512 * g + 512],
                                 start=True, stop=True)
                ptv_g = ptv_pool.tile([128, 512], f16, tag="ptv")
                nc.scalar.activation(ptv_g[:], stv[:], Exp,
                                     scale=float(SCALE))
                if g == 0:
                    # query block 0 must not see key block 1
                    nc.gpsimd.memset(ptv_g[64:128, 0:64], 0.0)
                ptvs[g] = ptv_g
                # banded scores: 4 quarters in one 2-bank PSUM tile,
                # only the live 192 of each 256 columns.
                st = st_ps.tile([128, 1024], f32, tag="stb")
                pts = pts_pool.tile([128, 1024], f16, tag="ptsb")
                pa = pts[:]
                if g < 3:
                    # one-time per ring slot: zero the dead 64-col block
                    # per quarter.  Nothing below ever writes them, so
                    # ring reuse keeps them zero.
                    gap = bass.AP(tensor=pa.tensor,
                                  offset=pa.offset + 192,
                                  ap=[[pa.ap[0][0], 128], [256, 4],
                                      [1, 64]])
                    nc.gpsimd.memset(gap, 0.0)
                q0 = 1 if g == 0 else 0
                for jj in range(q0, 4):
                    tt = 4 * g + jj
                    qw = min(192, SEQ - 128 * tt)
                    nc.tensor.matmul(
                        st[:, 256 * jj:256 * jj + qw],
                        kt[:, 128 * tt:128 * tt + 128],
                        qt[:, 128 * tt:128 * tt + qw],
                        start=True, stop=True)
                # one exp over the banded batch, skipping dead blocks
                nq = 4 - q0
                sa = st[:]
                src = bass.AP(tensor=sa.tensor,
                              offset=sa.offset + 256 * q0,
                              ap=[[sa.ap[0][0], 128], [256, nq], [1, 192]])
                dst = bass.AP(tensor=pa.tensor,
                              offset=pa.offset + 256 * q0,
                              ap=[[pa.ap[0][0], 128], [256, nq], [1, 192]])
                nc.scalar.activation(dst, src, Exp, scale=float(SCALE))
                # staircase corners for all quarters, 2 strided memsets:
                # k{2t} rows 0:64 invisible to q-block 2t+2 (cols
                # 128:192); k{2t+1} rows 64:128 invisible to q-block 2t
                # (cols 0:64).
                mA = bass.AP(tensor=pa.tensor,
                             offset=pa.offset + 256 * q0 + 128,
                             ap=[[pa.ap[0][0], 64], [256, nq], [1, 64]])
                nc.gpsimd.memset(mA, 0.0)
                mB = bass.AP(tensor=pa.tensor,
                             offset=pa.offset + 64 * pa.ap[0][0] + 256 * q0,
                             ap=[[pa.ap[0][0], 64], [256, nq], [1, 64]])
                nc.gpsimd.memset(mB, 0.0)
                pts_tiles[g] = pts

            make_scores(0)
            osb = None
            ovp = None

            for g in range(GROUPS):
                if g < 4:
                    # insurance against early load hiccups: keep the PE
                    # HAM-busy through the pipeline-fill phase (g=0 also
                    # bridges the exp(0)+memset latency before PV(0))
                    dummies(6 if g == 0 else 2, w=128)
                if g + 1 < GROUPS:
                    make_scores(g + 1)
                ptv = ptvs[g]
                pts = pts_tiles[g]
                for j in range(4):
                    t = 4 * g + j
                    qv = slice(128 * j, 128 * j + 128)

                    # PV: O'[q, 0:128]=O unnormalized, O'[q, 128]=denom.
                    # Two accumulators share a PSUM bank; 4 slots rotate.
                    if t % 2 == 0:
                        ovp = o_ps.tile([128, 2 * OW], f32, tag="ov")
                    ov = ovp[:, OW * (t % 2):OW * (t % 2) + OW]
                    nc.tensor.matmul(ov, ptv[:, qv], vbt(0),
                                     start=True, stop=(t == 0))
                    if t >= 2:
                        pprev = pts_tiles[(t - 1) // 4]
                        jprev = (t - 1) % 4
                        nc.tensor.matmul(ov,
                                         pprev[:, 256 * jprev + 128:
                                               256 * jprev + 256],
                                         vbt(t - 1), start=False,
                                         stop=False)
                    if t >= 1:
                        nc.tensor.matmul(ov,
                                         pts[:, 256 * j:256 * j + 128],
                                         vbt(t), start=False, stop=True)

                    # cast each finished pair PSUM -> SBUF fp16 (DVE),
                    # store every STORE_W iterations on the sync queue
                    sj = t % STORE_W
                    if sj == 0:
                        osb = o_pool.tile([128, OW * STORE_W], f16,
                                          tag="osb")
                    if t % 2 == 1:
                        half = OW * (sj - 1)
                        nc.vector.tensor_copy(
                            osb[:, half:half + 2 * OW], ovp[:])
                    if sj == STORE_W - 1:
                        t0 = t - STORE_W + 1
                        if g == GROUPS - 1:
                            # split the final store across both queues so
                            # the tail drains in parallel
                            nc.scalar.dma_start(
                                out=o_d[:, OW * t0:OW * t0 + 2 * OW],
                                in_=osb[:, 0:2 * OW])
                            nc.sync.dma_start(
                                out=o_d[:, OW * (t0 + 2):OW * (t0 + 4)],
                                in_=osb[:, 2 * OW:4 * OW])
                        else:
                            nc.sync.dma_start(
                                out=o_d[:, OW * t0:OW * t0 + OW * STORE_W],
                                in_=osb[:])


    nc.compile()
    return nc


def _get_nc():
    global _CACHED_NC
    if _CACHED_NC is None:
        _CACHED_NC = _build_nc()
    return _CACHED_NC


def _run(inputs, trace=False, trace_kwargs=None):
    from concourse.bass_utils import run_bass_kernel_spmd

    q, k, v = inputs["q"], inputs["k"], inputs["v"]
    block_mask = np.asarray(inputs["block_mask"])
    assert np.array_equal(block_mask, _expected_block_mask()), \
        "kernel compiled for the DKernel predefined sparse pattern only"

    nc = _get_nc()
    in_maps = []
    for h in range(N_CORES):
        qh = np.asarray(q[0, :, h, :], dtype=np.float32)
        kh = np.asarray(k[0, :, h, :], dtype=np.float32)
        vh = np.asarray(v[0, :, h, :], dtype=np.float32)
        # pre-tiled [V | 1] in [128, 32*129] layout: tile t holds V rows
        # [128t, 128t+128) with a trailing ones column
        vt = np.ones((128, TILES * VW), dtype=np.float16)
        vr = vh.astype(np.float16).reshape(TILES, 128, D)
        for t in range(TILES):
            vt[:, VW * t:VW * t + 128] = vr[t]
        in_maps.append({
            "qT": np.ascontiguousarray(qh.T.astype(np.float16)),
            "kT": np.ascontiguousarray(kh.T.astype(np.float16)),
            "vt": vt,
        })
    kwargs = {}
    if trace:
        kwargs["trace"] = True
        if trace_kwargs:
            kwargs.update(trace_kwargs)
    res = run_bass_kernel_spmd(nc, in_maps, list(range(N_CORES)), **kwargs)
    out = np.empty((1, SEQ, N_HEADS, D), dtype=np.float32)
    for h in range(N_CORES):
        r = np.asarray(res.results[h]["o"], dtype=np.float32)
        r = r.reshape(128, TILES, OW)
        num = r[:, :, 0:D].transpose(1, 0, 2).reshape(SEQ, D)
        den = r[:, :, D].transpose(1, 0).reshape(SEQ, 1)
        out[0, :, h, :] = num / den
    return out, res


def kernel(q, k, v, block_mask):
    out, _ = _run({"q": q, "k": k, "v": v, "block_mask": block_mask})
    return out


# revision 20
# speedup vs baseline: 1.0148x; 1.0148x over previous
"""Block-sparse attention kernel for TRN2 (8 NeuronCores, 1 head per core).

Problem: q,k,v [1, 4096, 8, 128] f32, block_mask [64,64] bool with pattern
  causal & (2-block sliding window | vertical stripe on blocks {0,1}).
Masking is block-granular (mask expanded by repeat), so active blocks are
fully dense.

Per-core strategy (one head). The host prepares fp16 operands:
  qT, kT: [128, 4096] transposed,  vt: [128, 32*129] pre-tiled V with a
  ones-column per 128-row tile, so P^T @ [V | 1] accumulates both O and
  the softmax denominators in one matmul chain.

Scores are computed TRANSPOSED (ST[k, q] = K @ Q^T) so exp(ST) directly
yields P^T — the stationary operand PV needs. No PE transposes at all.

Performance structure (vs the 44.4us v1 baseline -> ~32us):
  - Inputs land in three big SBUF tiles via chunked DMAs on BOTH HWDGE
    queues in need order (k/v stream on sync, q stream + v tail on
    scalar); Tile's subregion tracking means consumers only wait for
    the chunks they read.  The scalar-engine issue cost hides in the
    idle window before the first exp.
  - 8 dummy matmuls on scratch data (plus small fillers in the first
    loop bodies) keep the PE busy from t=0 so the HAM clock gate
    reaches 8/8 by ~3.5us and stays there (>=3.4us of continuous PE
    activity is required to un-throttle from the 1.2GHz cold clock).
  - Software pipeline: the loop body issues batch g+1's score matmuls
    + exp before batch g's PVs, so the single ACT engine (the pacing
    engine: ~13us of exp) always runs a batch ahead of its consumers.
  - Banded scores for 4 q-pair iterations accumulate in one [128,1024]
    PSUM tile (2 banks) and get ONE strided exp; matmuls and exp skip
    the dead 64-col block per 256 (q-pair t+3 never sees k{2t,2t+1}),
    and the staircase corners are zeroed with 2 strided memsets per
    batch instead of 2 per iteration.
  - Two [128,129] PV accumulators share each PSUM bank (pair tiles), so
    4 accumulation slots rotate and the PSUM->SBUF casts (one per pair,
    on DVE) never stall the PE.
  - Output is stored UNNORMALIZED as fp16 [O' | denom] tiles; the host
    divides.  Halves write traffic and removes the reciprocal chain.
    The final store is split across both queues so the tail drains in
    parallel.
Softmax skips max-subtraction: scores*scale ~ N(0,1), exp is safe
(denominators <= ~1.4e3, numerators <= ~2.5e3 — well inside fp16 range).
"""
import sys

if '/opt/trn_rl_repo' not in sys.path:
    sys.path.insert(0, '/opt/trn_rl_repo')

import numpy as np

SEQ = 4096
D = 128
BLOCK = 64
NBLK = SEQ // BLOCK
TILES = SEQ // 128           # 32 q-pair iterations
GROUPS = TILES // 4          # 8 batches (4 iterations each)
STORE_W = 4                  # iterations per output store
N_CORES = 8
N_HEADS = 8
SCALE = 1.0 / float(np.sqrt(D))
VW = 129                     # V tile width incl ones column
OW = 129                     # output tile width incl denominator column


def _expected_block_mask():
    q = np.arange(NBLK)[:, None]
    k = np.arange(NBLK)[None, :]
    causal = q >= k
    sliding = (q - k) < 2
    vert = np.zeros(NBLK, dtype=bool)
    vert[0:2] = True
    return causal & (sliding | vert[None, :])


_CACHED_NC = None


def _build_nc():
    import concourse.bass as bass
    import concourse.bacc as bacc
    import concourse.tile as tile
    import concourse.mybir as mybir

    f32 = mybir.dt.float32
    f16 = mybir.dt.float16
    Exp = mybir.ActivationFunctionType.Exp

    nc = bacc.Bacc(None, target_bir_lowering=False)

    qt_d = nc.dram_tensor("qT", [D, SEQ], f16, kind="ExternalInput")
    kt_d = nc.dram_tensor("kT", [D, SEQ], f16, kind="ExternalInput")
    v_d = nc.dram_tensor("vt", [D, TILES * VW], f16, kind="ExternalInput")
    o_d = nc.dram_tensor("o", [D, TILES * OW], f16, kind="ExternalOutput")

    with tile.TileContext(nc) as tc:
        with tc.tile_pool(name="inputs", bufs=1) as inputs, \
             tc.tile_pool(name="ptv_pool", bufs=3) as ptv_pool, \
             tc.tile_pool(name="pts_pool", bufs=3) as pts_pool, \
             tc.tile_pool(name="o_pool", bufs=5) as o_pool, \
             tc.tile_pool(name="stv_ps", bufs=2, space="PSUM") as stv_ps, \
             tc.tile_pool(name="st_ps", bufs=2, space="PSUM") as st_ps, \
             tc.tile_pool(name="o_ps", bufs=2, space="PSUM") as o_ps:

            kt = inputs.tile([128, SEQ], f16, name="kt", tag="kt")
            qt = inputs.tile([128, SEQ], f16, name="qt", tag="qt")
            vt = inputs.tile([128, TILES * VW], f16, name="vt", tag="vt")
            scr = inputs.tile([128, 512], f16, name="scr", tag="scr")

            # ---- loads: two HWDGE queues as parallel streams in need
            # order — the k stream (+ most of v) on sync, the q stream
            # (+ v tail) on scalar.  The scalar engine's issue cost all
            # lands in the idle fill phase before the first exp.  Tile
            # tracks subregions, so consumers only wait for the chunks
            # they actually read.
            nc.sync.dma_start(out=kt[:, 0:1024], in_=kt_d[:, 0:1024])
            nc.scalar.dma_start(out=qt[:, 0:576], in_=qt_d[:, 0:576])
            nc.sync.dma_start(out=vt[:, 0:516], in_=v_d[:, 0:516])
            nc.scalar.dma_start(out=qt[:, 576:1152], in_=qt_d[:, 576:1152])
            nc.sync.dma_start(out=kt[:, 1024:2048], in_=kt_d[:, 1024:2048])
            nc.scalar.dma_start(out=qt[:, 1152:2304], in_=qt_d[:, 1152:2304])
            nc.sync.dma_start(out=vt[:, 516:1032], in_=v_d[:, 516:1032])
            nc.scalar.dma_start(out=vt[:, 3096:4128], in_=v_d[:, 3096:4128])
            nc.sync.dma_start(out=vt[:, 1032:2064], in_=v_d[:, 1032:2064])
            nc.scalar.dma_start(out=qt[:, 2304:3456], in_=qt_d[:, 2304:3456])
            nc.sync.dma_start(out=kt[:, 2048:3072], in_=kt_d[:, 2048:3072])
            nc.scalar.dma_start(out=qt[:, 3456:4096], in_=qt_d[:, 3456:4096])
            nc.sync.dma_start(out=vt[:, 2064:3096], in_=v_d[:, 2064:3096])
            nc.sync.dma_start(out=kt[:, 3072:4096], in_=kt_d[:, 3072:4096])

            nc.gpsimd.memset(scr[:], 0.0)

            def vbt(t):
                return vt[:, VW * t:VW * t + VW]

            # ---- PE warm-up: dummy matmuls keep the PE busy from t=0 so
            # the HAM clock gate reaches 8/8 by ~3.5us while the first
            # input chunks are in flight.  They cycle through the stv
            # PSUM ring (write-only; in-order WAW deps are free).
            def dummies(n, w=512):
                for _ in range(n):
                    dummy = stv_ps.tile([128, 512], f32,
                                        name="dummy", tag="stv")
                    nc.tensor.matmul(dummy[:, 0:w], scr[:, 0:128],
                                     scr[:, 0:w], start=True, stop=True)

            dummies(8)

            ptvs = [None] * GROUPS
            pts_tiles = [None] * GROUPS   # [128, 1024] fp16, 4 quarters

            def make_scores(g):
                """Score matmuls + exp + corner memsets for batch g."""
                # vertical stripe for this group's 512 q columns
                stv = stv_ps.tile([128, 512], f32, name="stv", tag="stv")
                nc.tensor.matmul(stv[:], kt[:, 0:128],
                                 qt[:, 512 * g:512 * g + 512],
                                 start=True, stop=True)
                ptv_g = ptv_pool.tile([128, 512], f16, tag="ptv")
                nc.scalar.activation(ptv_g[:], stv[:], Exp,
                                     scale=float(SCALE))
                if g == 0:
                    # query block 0 must not see key block 1
                    nc.gpsimd.memset(ptv_g[64:128, 0:64], 0.0)
                ptvs[g] = ptv_g
                # banded scores: 4 quarters in one 2-bank PSUM tile,
                # only the live 192 of each 256 columns.
                st = st_ps.tile([128, 1024], f32, tag="stb")
                pts = pts_pool.tile([128, 1024], f16, tag="ptsb")
                pa = pts[:]
                if g < 3:
                    # one-time per ring slot: zero the dead 64-col block
                    # per quarter.  Nothing below ever writes them, so
                    # ring reuse keeps them zero.
                    gap = bass.AP(tensor=pa.tensor,
                                  offset=pa.offset + 192,
                                  ap=[[pa.ap[0][0], 128], [256, 4],
                                      [1, 64]])
                    nc.gpsimd.memset(gap, 0.0)
                q0 = 1 if g == 0 else 0
                for jj in range(q0, 4):
                    tt = 4 * g + jj
                    qw = min(192, SEQ - 128 * tt)
                    nc.tensor.matmul(
                        st[:, 256 * jj:256 * jj + qw],
                        kt[:, 128 * tt:128 * tt + 128],
                        qt[:, 128 * tt:128 * tt + qw],
                        start=True, stop=True)
                # one exp over the banded batch, skipping dead blocks
                nq = 4 - q0
                sa = st[:]
                src = bass.AP(tensor=sa.tensor,
                              offset=sa.offset + 256 * q0,
                              ap=[[sa.ap[0][0], 128], [256, nq], [1, 192]])
                dst = bass.AP(tensor=pa.tensor,
                              offset=pa.offset + 256 * q0,
                              ap=[[pa.ap[0][0], 128], [256, nq], [1, 192]])
                nc.scalar.activation(dst, src, Exp, scale=float(SCALE))
                # staircase corners for all quarters, 2 strided memsets:
                # k{2t} rows 0:64 invisible to q-block 2t+2 (cols
                # 128:192); k{2t+1} rows 64:128 invisible to q-block 2t
                # (cols 0:64).
                mA = bass.AP(tensor=pa.tensor,
                             offset=pa.offset + 256 * q0 + 128,
                             ap=[[pa.ap[0][0], 64], [256, nq], [1, 64]])
                nc.gpsimd.memset(mA, 0.0)
                mB = bass.AP(tensor=pa.tensor,
                             offset=pa.offset + 64 * pa.ap[0][0] + 256 * q0,
                             ap=[[pa.ap[0][0], 64], [256, nq], [1, 64]])
                nc.gpsimd.memset(mB, 0.0)
                pts_tiles[g] = pts

            make_scores(0)
            osb = None
            ovp = None

            for g in range(GROUPS):
                if g == 0:
                    # bridge the exp(0)+memset latency before PV(0) so
                    # the PE stays HAM-busy through the pipeline fill
                    dummies(6, w=128)
                if g + 1 < GROUPS:
                    make_scores(g + 1)
                ptv = ptvs[g]
                pts = pts_tiles[g]
                for j in range(4):
                    t = 4 * g + j
                    qv = slice(128 * j, 128 * j + 128)

                    # PV: O'[q, 0:128]=O unnormalized, O'[q, 128]=denom.
                    # Two accumulators share a PSUM bank; 4 slots rotate.
                    if t % 2 == 0:
                        ovp = o_ps.tile([128, 2 * OW], f32, tag="ov")
                    ov = ovp[:, OW * (t % 2):OW * (t % 2) + OW]
                    nc.tensor.matmul(ov, ptv[:, qv], vbt(0),
                                     start=True, stop=(t == 0))
                    if t >= 2:
                        pprev = pts_tiles[(t - 1) // 4]
                        jprev = (t - 1) % 4
                        nc.tensor.matmul(ov,
                                         pprev[:, 256 * jprev + 128:
                                               256 * jprev + 256],
                                         vbt(t - 1), start=False,
                                         stop=False)
                    if t >= 1:
                        nc.tensor.matmul(ov,
                                         pts[:, 256 * j:256 * j + 128],
                                         vbt(t), start=False, stop=True)

                    # cast each finished pair PSUM -> SBUF fp16 (DVE),
                    # store every STORE_W iterations on the sync queue
                    sj = t % STORE_W
                    if sj == 0:
                        osb = o_pool.tile([128, OW * STORE_W], f16,
                                          tag="osb")
                    if t % 2 == 1:
                        half = OW * (sj - 1)
                        nc.vector.tensor_copy(
                            osb[:, half:half + 2 * OW], ovp[:])
                    if sj == STORE_W - 1:
                        t0 = t - STORE_W + 1
                        if g == GROUPS - 1:
                            # split the final store across both queues so
                            # the tail drains in parallel
                            nc.scalar.dma_start(
                                out=o_d[:, OW * t0:OW * t0 + 2 * OW],
                                in_=osb[:, 0:2 * OW])
                            nc.sync.dma_start(
                                out=o_d[:, OW * (t0 + 2):OW * (t0 + 4)],
                                in_=osb[:, 2 * OW:4 * OW])
                        else:
                            nc.sync.dma_start(
                                out=o_d[:, OW * t0:OW * t0 + OW * STORE_W],
                                in_=osb[:])


    nc.compile()
    return nc


def _get_nc():
    global _CACHED_NC
    if _CACHED_NC is None:
        _CACHED_NC = _build_nc()
    return _CACHED_NC


def _run(inputs, trace=False, trace_kwargs=None):
    from concourse.bass_utils import run_bass_kernel_spmd

    q, k, v = inputs["q"], inputs["k"], inputs["v"]
    block_mask = np.asarray(inputs["block_mask"])
    assert np.array_equal(block_mask, _expected_block_mask()), \
        "kernel compiled for the DKernel predefined sparse pattern only"

    nc = _get_nc()
    in_maps = []
    for h in range(N_CORES):
        qh = np.asarray(q[0, :, h, :], dtype=np.float32)
        kh = np.asarray(k[0, :, h, :], dtype=np.float32)
        vh = np.asarray(v[0, :, h, :], dtype=np.float32)
        # pre-tiled [V | 1] in [128, 32*129] layout: tile t holds V rows
        # [128t, 128t+128) with a trailing ones column
        vt = np.ones((128, TILES * VW), dtype=np.float16)
        vr = vh.astype(np.float16).reshape(TILES, 128, D)
        for t in range(TILES):
            vt[:, VW * t:VW * t + 128] = vr[t]
        in_maps.append({
            "qT": np.ascontiguousarray(qh.T.astype(np.float16)),
            "kT": np.ascontiguousarray(kh.T.astype(np.float16)),
            "vt": vt,
        })
    kwargs = {}
    if trace:
        kwargs["trace"] = True
        if trace_kwargs:
            kwargs.update(trace_kwargs)
    res = run_bass_kernel_spmd(nc, in_maps, list(range(N_CORES)), **kwargs)
    out = np.empty((1, SEQ, N_HEADS, D), dtype=np.float32)
    for h in range(N_CORES):
        r = np.asarray(res.results[h]["o"], dtype=np.float32)
        r = r.reshape(128, TILES, OW)
        num = r[:, :, 0:D].transpose(1, 0, 2).reshape(SEQ, D)
        den = r[:, :, D].transpose(1, 0).reshape(SEQ, 1)
        out[0, :, h, :] = num / den
    return out, res


def kernel(q, k, v, block_mask):
    out, _ = _run({"q": q, "k": k, "v": v, "block_mask": block_mask})
    return out
